# revision 30
# baseline (speedup 1.0000x reference)
"""Multi-head attention (B=4, S=2048, D=768, H=12) on 8 Trainium2 cores.

Sharding: the 48 (batch, head) pairs are data-parallel; each core gets 6.

Per head on one core (matmuls bf16, fp32 PSUM accumulation):
  Host folds the Q/K projections into one: s_qk = x̃_q·u_k + c_k with
    u = Ŵuᵀ x̃  (Ŵu = [[WqᵀWk]; (Wqᵀbk)ᵀ], x̃ = [x; 1], K=65)
    c = ŵcᵀ x̃  (ŵc = [Wkᵀbq; bqᵀbk]/8, folded into the V projection)
  so only u (not q and k) is computed on device, and the per-k score bias
  c rides the exp as a per-partition bias operand.
  uT [128, S]  : u duplicated into both partition halves directly by the
                 projection (two col-tiled matmuls) so score matmuls can
                 row-pair: two K=64 matmuls concurrent in the PE array
                 (lhsT = uT halves, rhs = x duplicated via double-DMA).
  V    [S,64+1]: ones column appended -> AV matmul also produces the
                 softmax denominator. V projection (lhsT = x̃ chunks,
                 rhs = [WvT;bv | 0 | ŵc]) emits V, the ones placeholder,
                 and the c column in one FD=66 matmul per 128 positions.
  scoresT [k,q]: per 128-row k-chunk, [128, 1024] PSUM tiles; score
                 matmuls ordered A,B,A,B so the two row-halves stream
                 concurrently in the PE array.
  P = exp(s/8 + c): split between ACT (exact, activation bias=c) and DVE
                 (Schraudolph bits: int16(s*A + c2) reinterpreted as bf16,
                 ~3% max rel err) so neither engine is the wall.
  out^T [65,512] = sum_k V_aug^T P; row 64 = softmax denominator.
                 Denominator rows bounce through DRAM and return as one
                 [128, 16] tile for a single cheap reciprocal; a 0-stride
                 DMA re-read broadcasts each reciprocal row across
                 partitions; the normalize multiply runs on GPSIMD to keep
                 the DVE free for exp. Output lands in [e, q] layout; the
                 host gather transposes it back.

Scheduling: score-pair emission is interleaved with filler PE work (AV
matmuls of the previous q-block, projections of the next head) via a FIFO
of generators, keeping tensor/ACT/DVE all dense.
"""

import sys
from collections import deque

for _p in ("/opt/trn_rl_repo",):
    if _p not in sys.path:
        sys.path.insert(0, _p)

import numpy as np

B, S, D, H = 4, 2048, 768, 12
DH = 64
NCORES = 8
HPC = (B * H) // NCORES  # 6 heads per core
SCALE = 1.0 / 8.0
NKC = S // 128  # 16 k-chunks
NQB = 2  # q blocks of 1024
QB = S // NQB
PUMPS_PER_PAIR = 5

# Schraudolph exp-as-bf16-bits on the DVE: bits_i16 = round(s*EXPA + c2)
# approximates bf16(exp(s/8 + c)) to ~3% max rel err; offloads the ACT engine.
LOG2E128 = 128.0 / np.log(2.0)
EXPA = LOG2E128 / 8.0  # includes the 1/8 softmax scale
EXPB = (127.0 - 0.0436) * 128.0
# tile indices (kc*2 + half) handled by DVE instead of ACT, per (head, jb)
DVE_EXP = frozenset({1, 3, 5, 7, 9, 11, 13, 15})


def _split_multi_waits(nc):
    """This walrus build rejects >1 sync wait per instruction. Insert
    single-wait NoOps (same engine, so same instruction stream) ahead of
    any instruction carrying several waits."""
    import bass_rust
    import concourse.mybir as mybir

    n_split = 0
    for f in nc.m.functions:
        for bb in f.blocks:
            out = []
            dirty = False
            for inst in bb.instructions:
                si = inst.sync_info
                if si is not None and len(si.on_wait) > 1:
                    waits = list(si.on_wait)
                    for j, w in enumerate(waits[:-1]):
                        nop = mybir.InstNoOp(name=f"{inst.name}-w{j}", ins=[], outs=[])
                        nop.engine = inst.engine
                        nop.sync_info = bass_rust.SyncInfo(on_wait=[w], on_update=[])
                        out.append(nop)
                    si.on_wait = waits[-1:]
                    dirty = True
                    n_split += 1
                out.append(inst)
            if dirty:
                bb.instructions = out
    return n_split


_BUILT = None


def build():
    global _BUILT
    if _BUILT is not None:
        return _BUILT
    import concourse.bass as bass
    import concourse.mybir as mybir
    import concourse.tile as tile

    F32 = mybir.dt.float32
    BF = mybir.dt.bfloat16
    I16 = mybir.dt.int16
    AF = mybir.ActivationFunctionType
    ALU = mybir.AluOpType

    nc = bass.Bass()
    xtd = nc.dram_tensor("xt", [HPC, 65, S], BF, kind="ExternalInput")
    wud = nc.dram_tensor("wu", [HPC, 65, 64], BF, kind="ExternalInput")
    wvcd = nc.dram_tensor("wvc", [HPC, 65, 66], BF, kind="ExternalInput")
    outd = nc.dram_tensor("out", [HPC, 64, S], F32, kind="ExternalOutput")
    dnd = nc.dram_tensor("dnd", [HPC, NQB, QB], F32)  # denominator bounce
    rcd = nc.dram_tensor("rcd", [HPC, NQB, QB], F32)  # reciprocal bounce

    # V-proj psum grouping: 16 chunks in groups of 7/7/2 (66 cols each)
    VGRP = ((0, 7), (7, 14), (14, 16))

    with tile.TileContext(nc) as tc:
        with (
            tc.tile_pool(name="x", bufs=2) as xpool,
            tc.tile_pool(name="w", bufs=2) as wpool,
            tc.tile_pool(name="u", bufs=2) as upool,
            tc.tile_pool(name="v", bufs=2) as vpool,
            tc.tile_pool(name="pt", bufs=2 * NKC * NQB) as ptpool,
            tc.tile_pool(name="ot", bufs=9) as otpool,
            tc.tile_pool(name="r", bufs=3) as rpool,
            tc.tile_pool(name="sp", bufs=3, space="PSUM") as sppool,
            tc.tile_pool(name="avp", bufs=2, space="PSUM") as avpool,
        ):
            state = {}

            def proj_steps(i):
                # u-proj dependencies issue first (and on the sync queue) so
                # head-0 matmuls start as early as possible
                xt = xpool.tile([65, S], BF, tag="xt", name=f"xt{i}")
                nc.sync.dma_start(xt[0:65, :], xtd[i])
                wu = wpool.tile([65, 64], BF, tag="wu", name=f"wu{i}")
                nc.sync.dma_start(wu[0:65, :], wud[i])
                xx = xpool.tile([128, S], BF, tag="xx", name=f"xx{i}")
                nc.gpsimd.dma_start(xx[0:64, :], xtd[i, 0:64])
                nc.gpsimd.dma_start(xx[64:128, :], xtd[i, 0:64])
                wvc = wpool.tile([65, 66], BF, tag="wvc", name=f"wvc{i}")
                nc.gpsimd.dma_start(wvc[0:65, :], wvcd[i])
                yield

                # u projection, duplicated into both partition halves by
                # col-tiled matmul pairs (concurrent in the PE array)
                ut = upool.tile([128, S], BF, tag="u", name=f"u{i}")
                for ch in range(4):
                    ps = sppool.tile([128, 512], F32, tag="sp", name=f"up{i}_{ch}")
                    rhs = xt[0:65, ch * 512 : (ch + 1) * 512]
                    nc.tensor.matmul(ps[0:64, :], wu[0:65, :], rhs)
                    nc.tensor.matmul(ps[64:128, :], wu[0:65, :], rhs)
                    nc.scalar.copy(ut[:, ch * 512 : (ch + 1) * 512], ps[:])
                    yield

                # V projection: per 128-position chunk one FD=66 matmul
                # emitting [V (64) | 0 (ones placeholder) | c], grouped so one
                # PSUM->SBUF copy covers up to 7 chunks.
                v_sb = vpool.tile([128, NKC, 66], BF, tag="v", name=f"v{i}")
                for k0, k1 in VGRP:
                    nch = k1 - k0
                    ps = sppool.tile([128, 462], F32, tag="sp", name=f"vp{i}_{k0}")
                    for j in range(nch):
                        kc = k0 + j
                        nc.tensor.matmul(
                            ps[:, j * 66 : (j + 1) * 66],
                            xt[0:65, kc * 128 : (kc + 1) * 128],
                            wvc[0:65, :],
                        )
                    nc.scalar.copy(
                        v_sb[:, k0:k1, :].rearrange("p a b -> p (a b)"),
                        ps[:, 0 : nch * 66],
                    )
                    yield
                nc.gpsimd.memset(v_sb[:, :, 64:65], 1.0)
                # DVE Schraudolph per-partition bias: c2 = c*LOG2E128 + EXPB
                c2 = rpool.tile([128, NKC], F32, tag="c2", name=f"c2{i}")
                nc.vector.tensor_scalar(
                    c2[:], v_sb[:, :, 65], LOG2E128, EXPB, ALU.mult, ALU.add
                )
                yield
                state[i] = {"xx": xx, "u": ut, "v": v_sb, "c2": c2, "pt": {}}

            def sc_pair(i, jb, kc):
                """One kc-pair of row-tiled score matmuls + their exps."""
                st = state[i]
                xx, ut, v_sb, c2 = st["xx"], st["u"], st["v"], st["c2"]
                pt = st["pt"].setdefault(jb, [None] * NKC)
                tA = sppool.tile([128, QB], F32, tag="sp", name=f"sA{i}_{jb}_{kc}")
                tB = sppool.tile([128, QB], F32, tag="sp", name=f"sB{i}_{jb}_{kc}")
                lA = ut[0:64, kc * 128 : (kc + 1) * 128]
                lB = ut[64:128, (kc + 8) * 128 : (kc + 9) * 128]
                # A/B interleaved: the two row-halves stream concurrently
                for qm in range(QB // 512):
                    q0 = jb * QB + qm * 512
                    sl = slice(qm * 512, (qm + 1) * 512)
                    nc.tensor.matmul(tA[:, sl], lA, xx[0:64, q0 : q0 + 512])
                    nc.tensor.matmul(tB[:, sl], lB, xx[64:128, q0 : q0 + 512])
                pA = ptpool.tile([128, QB], BF, tag="pt", name=f"pA{i}_{jb}_{kc}")
                pB = ptpool.tile([128, QB], BF, tag="pt", name=f"pB{i}_{jb}_{kc}")
                for half, (p, t, c) in enumerate(((pA, tA, kc), (pB, tB, kc + 8))):
                    if kc * 2 + half in DVE_EXP:
                        nc.vector.tensor_scalar(
                            p[:].bitcast(I16),
                            t[:],
                            EXPA,
                            c2[:, c : c + 1],
                            ALU.mult,
                            ALU.add,
                        )
                    else:
                        nc.scalar.activation(
                            p[:], t[:], AF.Exp, bias=v_sb[:, c, 65:66], scale=SCALE
                        )
                pt[kc] = pA
                pt[kc + 8] = pB

            def av_steps(i, jb):
                """Generator: AV matmuls in groups of 4; numerators parked in
                SBUF, denominator rows bounced to DRAM. After the last
                q-chunk of the q-block: one [128,8] reciprocal, bounce back,
                broadcast-read per chunk, GPSIMD multiply, store."""
                v_sb = state[i]["v"]
                pt = state[i]["pt"].pop(jb)
                ots_list = []
                # kc-outer / qm-inner: consecutive matmul pairs share the same
                # V weights, keeping every LDWEIGHTS hidden under a stream
                avs = [
                    avpool.tile([128, 512], F32, tag="av", name=f"av{i}_{jb}_{qm}")
                    for qm in range(QB // 512)
                ]
                for kc in range(NKC):
                    for qm in range(QB // 512):
                        nc.tensor.matmul(
                            avs[qm][0:65, :],
                            v_sb[:, kc, 0:65],
                            pt[kc][:, qm * 512 : (qm + 1) * 512],
                            start=(kc == 0),
                            stop=(kc == NKC - 1),
                        )
                    if kc % 2 == 1 and kc < NKC - 1:
                        yield
                for qm, av in enumerate(avs):
                    # drain copy emitted immediately after the last AV matmul
                    # so it queues ahead of subsequently-emitted exp work
                    ots = otpool.tile([65, 512], F32, tag="ot", name=f"ot{i}_{jb}_{qm}")
                    if i == HPC - 1:
                        # tail fast path: 1-lane recip straight from the AV
                        # PSUM row so the normalize chain starts immediately
                        r1 = rpool.tile([1, 512], F32, tag="r1", name=f"r1{i}_{jb}_{qm}")
                        nc.vector.reciprocal(r1[:], av[64:65, :])
                        nc.sync.dma_start(
                            rcd[i, jb, qm * 512 : (qm + 1) * 512].rearrange(
                                "(a n) -> a n", a=1
                            ),
                            r1[:],
                        )
                    nc.vector.tensor_copy(ots[:], av[0:65, :])
                    if i < HPC - 1:
                        nc.sync.dma_start(
                            dnd[i, jb, qm * 512 : (qm + 1) * 512].rearrange(
                                "(a n) -> a n", a=1
                            ),
                            ots[64:65, :],
                        )
                    ots_list.append(ots)
                    yield
                import concourse.mybir as mybir

                if i < HPC - 1:
                    den = rpool.tile([128, 8], F32, tag="r", name=f"dn{i}_{jb}")
                    nc.sync.dma_start(
                        den[:], dnd[i, jb].rearrange("(p c) -> p c", c=8)
                    )
                    r8 = rpool.tile([128, 8], F32, tag="r", name=f"rc{i}_{jb}")
                    nc.vector.reciprocal(r8[:], den[:])
                    nc.sync.dma_start(
                        rcd[i, jb].rearrange("(p c) -> p c", c=8), r8[:]
                    )
                    yield
                for qm, ots in enumerate(ots_list):
                    g = jb * (QB // 512) + qm
                    rb = rpool.tile([64, 512], F32, tag="rb", name=f"rb{i}_{jb}_{qm}")
                    nc.sync.dma_start(
                        rb[:],
                        rcd[i, jb, qm * 512 : (qm + 1) * 512]
                        .rearrange("(a n) -> a n", a=1)
                        .to_broadcast((64, 512)),
                    )
                    nc.gpsimd.tensor_tensor(
                        ots[0:64, :], ots[0:64, :], rb[:], mybir.AluOpType.mult
                    )
                    nc.sync.dma_start(
                        outd[i][:, g * 512 : (g + 1) * 512], ots[0:64, :]
                    )
                    yield

            fillers = deque()

            def pump(n):
                while n > 0 and fillers:
                    try:
                        next(fillers[0])
                        n -= 1
                    except StopIteration:
                        fillers.popleft()

            def drain(gen=None):
                while fillers and (gen is None or gen in fillers):
                    pump(1)

            def unit(i, jb):
                # score pairs emitted in bursts of two: fewer PE phase
                # transitions (each score<->AV switch costs a ~300ns bubble)
                for kc in range(0, NKC // 2, 8):
                    for k2 in range(8):
                        sc_pair(i, jb, kc + k2)
                    pump(8 * PUMPS_PER_PAIR)

            # head 0 projections run eagerly; afterwards proj(i+1) + AV trail
            # the score stream as interleaved filler, lagging by one q-block
            g0 = proj_steps(0)
            fillers.append(g0)
            drain(g0)
            unit(0, 0)
            for i in range(HPC):
                if i > 0:
                    fillers.append(av_steps(i - 1, 1))
                    unit(i, 0)
                fillers.append(av_steps(i, 0))
                if i + 1 < HPC:
                    g = proj_steps(i + 1)
                    fillers.append(g)
                    unit(i, 1)
                    drain(g)
                else:
                    unit(i, 1)
            fillers.append(av_steps(HPC - 1, 1))
            drain()

    _split_multi_waits(nc)
    _BUILT = nc
    return nc


def _core_inputs(sequences, wq, bq, wk, bk, wv, bv):
    import ml_dtypes

    bf16 = ml_dtypes.bfloat16
    xh = np.asarray(sequences, dtype=np.float32).reshape(B, S, H, DH)
    wq, bq = np.asarray(wq, np.float32), np.asarray(bq, np.float32)
    wk, bk = np.asarray(wk, np.float32), np.asarray(bk, np.float32)
    wv, bv = np.asarray(wv, np.float32), np.asarray(bv, np.float32)
    in_maps = []
    for c in range(NCORES):
        xt = np.empty((HPC, 65, S), dtype=bf16)
        wu = np.empty((HPC, 65, 64), dtype=bf16)
        wvc = np.zeros((HPC, 65, 66), dtype=bf16)
        for i in range(HPC):
            f = c * HPC + i
            b, h = f // H, f % H
            xt[i, 0:64] = np.ascontiguousarray(xh[b, :, h, :].T).astype(bf16)
            xt[i, 64] = np.float32(1.0)
            # u = Wu^T xt:  Wu = [[ (Wq^T Wk)^T ]; (Wq^T bk)^T ] as [65, 64]
            M = wq[h].T @ wk[h]  # [a, b]
            g = wq[h].T @ bk[h]  # [a]
            wu[i, 0:64] = M.T.astype(bf16)
            wu[i, 64] = g.astype(bf16)
            # V projection rhs: [Wv^T; bv] in cols 0:64, zeros col 64 (ones
            # placeholder), c-weights col 65 (pre-scaled by 1/8)
            wvc[i, 0:64, 0:64] = wv[h].T.astype(bf16)
            wvc[i, 64, 0:64] = bv[h].astype(bf16)
            wc = np.concatenate([wk[h].T @ bq[h], [bq[h] @ bk[h]]]) * SCALE
            wvc[i, :, 65] = wc.astype(bf16)
        in_maps.append({"xt": xt, "wu": wu, "wvc": wvc})
    return in_maps


def _gather(results):
    out = np.empty((B, S, H, DH), np.float32)
    for c in range(NCORES):
        o = np.asarray(results[c]["out"])  # [HPC, 64, S]
        for i in range(HPC):
            f = c * HPC + i
            b, h = f // H, f % H
            out[b, :, h, :] = o[i].T
    return out.reshape(B, S, D)


def kernel(sequences, wq, bq, wk, bk, wv, bv):
    from concourse.bass_utils import run_bass_kernel_spmd

    nc = build()
    in_maps = _core_inputs(sequences, wq, bq, wk, bk, wv, bv)
    res = run_bass_kernel_spmd(nc, in_maps, list(range(NCORES)))
    return _gather(res.results)


# revision 32
# speedup vs baseline: 1.0462x; 1.0462x over previous
"""Multi-head attention (B=4, S=2048, D=768, H=12) on 8 Trainium2 cores.

Sharding: the 48 (batch, head) pairs are data-parallel; each core gets 6.

Per head on one core (matmuls bf16, fp32 PSUM accumulation):
  Host folds the Q/K projections into one: s_qk = x̃_q·u_k + c_k with
    u = Ŵuᵀ x̃  (Ŵu = [[WqᵀWk]; (Wqᵀbk)ᵀ], x̃ = [x; 1], K=65)
    c = ŵcᵀ x̃  (ŵc = [Wkᵀbq; bqᵀbk]/8, folded into the V projection)
  so only u (not q and k) is computed on device, and the per-k score bias
  c rides the exp as a per-partition bias operand.
  uT [128, S]  : u duplicated into both partition halves directly by the
                 projection (two col-tiled matmuls) so score matmuls can
                 row-pair: two K=64 matmuls concurrent in the PE array
                 (lhsT = uT halves, rhs = x duplicated via double-DMA).
  V    [S,64+1]: ones column appended -> AV matmul also produces the
                 softmax denominator. V projection (lhsT = x̃ chunks,
                 rhs = [WvT;bv | 0 | ŵc]) emits V, the ones placeholder,
                 and the c column in one FD=66 matmul per 128 positions.
  scoresT [k,q]: per 128-row k-chunk, [128, 1024] PSUM tiles; score
                 matmuls ordered A,B,A,B so the two row-halves stream
                 concurrently in the PE array.
  P = exp(s/8 + c): split between ACT (exact, activation bias=c) and DVE
                 (Schraudolph bits: int16(s*A + c2) reinterpreted as bf16,
                 ~3% max rel err) so neither engine is the wall.
  out^T [65,512] = sum_k V_aug^T P; row 64 = softmax denominator.
                 Denominator rows bounce through DRAM and return as one
                 [128, 16] tile for a single cheap reciprocal; a 0-stride
                 DMA re-read broadcasts each reciprocal row across
                 partitions; the normalize multiply runs on GPSIMD to keep
                 the DVE free for exp. Output lands in [e, q] layout; the
                 host gather transposes it back.

Scheduling: score-pair emission is interleaved with filler PE work (AV
matmuls of the previous q-block, projections of the next head) via a FIFO
of generators, keeping tensor/ACT/DVE all dense.
"""

import sys
from collections import deque

for _p in ("/opt/trn_rl_repo",):
    if _p not in sys.path:
        sys.path.insert(0, _p)

import numpy as np

B, S, D, H = 4, 2048, 768, 12
DH = 64
NCORES = 8
HPC = (B * H) // NCORES  # 6 heads per core
SCALE = 1.0 / 8.0
NKC = S // 128  # 16 k-chunks
NQB = 2  # q blocks of 1024
QB = S // NQB
PUMPS_PER_PAIR = 5

# Schraudolph exp-as-bf16-bits on the DVE: bits_i16 = round(s*EXPA + c2)
# approximates bf16(exp(s/8 + c)) to ~3% max rel err; offloads the ACT engine.
LOG2E128 = 128.0 / np.log(2.0)
EXPA = LOG2E128 / 8.0  # includes the 1/8 softmax scale
EXPB = (127.0 - 0.0436) * 128.0
# tile indices (kc*2 + half) handled by DVE instead of ACT, per (head, jb)
DVE_EXP = frozenset({1, 3, 5, 7, 9, 11, 13, 15})


def _split_multi_waits(nc):
    """This walrus build rejects >1 sync wait per instruction. Insert
    single-wait NoOps (same engine, so same instruction stream) ahead of
    any instruction carrying several waits."""
    import bass_rust
    import concourse.mybir as mybir

    n_split = 0
    for f in nc.m.functions:
        for bb in f.blocks:
            out = []
            dirty = False
            for inst in bb.instructions:
                si = inst.sync_info
                if si is not None and len(si.on_wait) > 1:
                    waits = list(si.on_wait)
                    for j, w in enumerate(waits[:-1]):
                        nop = mybir.InstNoOp(name=f"{inst.name}-w{j}", ins=[], outs=[])
                        nop.engine = inst.engine
                        nop.sync_info = bass_rust.SyncInfo(on_wait=[w], on_update=[])
                        out.append(nop)
                    si.on_wait = waits[-1:]
                    dirty = True
                    n_split += 1
                out.append(inst)
            if dirty:
                bb.instructions = out
    return n_split


_BUILT = None


def build():
    global _BUILT
    if _BUILT is not None:
        return _BUILT
    import concourse.bass as bass
    import concourse.mybir as mybir
    import concourse.tile as tile

    F32 = mybir.dt.float32
    BF = mybir.dt.bfloat16
    I16 = mybir.dt.int16
    AF = mybir.ActivationFunctionType
    ALU = mybir.AluOpType

    nc = bass.Bass()
    xtd = nc.dram_tensor("xt", [HPC, 65, S], BF, kind="ExternalInput")
    wud = nc.dram_tensor("wu", [HPC, 65, 64], BF, kind="ExternalInput")
    wvcd = nc.dram_tensor("wvc", [HPC, 65, 66], BF, kind="ExternalInput")
    outd = nc.dram_tensor("out", [HPC, 64, S], F32, kind="ExternalOutput")
    dnd = nc.dram_tensor("dnd", [HPC, NQB, QB], F32)  # denominator bounce
    rcd = nc.dram_tensor("rcd", [HPC, NQB, QB], F32)  # reciprocal bounce

    # V-proj psum grouping: 16 chunks in groups of 7/7/2 (66 cols each)
    VGRP = ((0, 7), (7, 14), (14, 16))

    with tile.TileContext(nc) as tc:
        with (
            tc.tile_pool(name="x", bufs=2) as xpool,
            tc.tile_pool(name="w", bufs=2) as wpool,
            tc.tile_pool(name="u", bufs=2) as upool,
            tc.tile_pool(name="v", bufs=2) as vpool,
            tc.tile_pool(name="pt", bufs=2 * NKC * NQB) as ptpool,
            tc.tile_pool(name="ot", bufs=9) as otpool,
            tc.tile_pool(name="r", bufs=3) as rpool,
            tc.tile_pool(name="sp", bufs=3, space="PSUM") as sppool,
            tc.tile_pool(name="avp", bufs=2, space="PSUM") as avpool,
        ):
            state = {}

            def proj_steps(i):
                # u-proj dependencies issue first (and on the sync queue) so
                # head-0 matmuls start as early as possible
                xt = xpool.tile([65, S], BF, tag="xt", name=f"xt{i}")
                nc.sync.dma_start(xt[0:65, :], xtd[i])
                wu = wpool.tile([65, 64], BF, tag="wu", name=f"wu{i}")
                nc.sync.dma_start(wu[0:65, :], wud[i])
                xx = xpool.tile([128, S], BF, tag="xx", name=f"xx{i}")
                nc.gpsimd.dma_start(xx[0:64, :], xtd[i, 0:64])
                nc.gpsimd.dma_start(xx[64:128, :], xtd[i, 0:64])
                wvc = wpool.tile([65, 66], BF, tag="wvc", name=f"wvc{i}")
                nc.gpsimd.dma_start(wvc[0:65, :], wvcd[i])
                yield

                # u projection, duplicated into both partition halves by
                # col-tiled matmul pairs (concurrent in the PE array)
                ut = upool.tile([128, S], BF, tag="u", name=f"u{i}")
                for ch in range(4):
                    ps = sppool.tile([128, 512], F32, tag="sp", name=f"up{i}_{ch}")
                    rhs = xt[0:65, ch * 512 : (ch + 1) * 512]
                    nc.tensor.matmul(ps[0:64, :], wu[0:65, :], rhs)
                    nc.tensor.matmul(ps[64:128, :], wu[0:65, :], rhs)
                    nc.scalar.copy(ut[:, ch * 512 : (ch + 1) * 512], ps[:])
                    yield

                # V projection: per 128-position chunk one FD=66 matmul
                # emitting [V (64) | 0 (ones placeholder) | c], grouped so one
                # PSUM->SBUF copy covers up to 7 chunks.
                v_sb = vpool.tile([128, NKC, 66], BF, tag="v", name=f"v{i}")
                for k0, k1 in VGRP:
                    nch = k1 - k0
                    ps = sppool.tile([128, 462], F32, tag="sp", name=f"vp{i}_{k0}")
                    for j in range(nch):
                        kc = k0 + j
                        nc.tensor.matmul(
                            ps[:, j * 66 : (j + 1) * 66],
                            xt[0:65, kc * 128 : (kc + 1) * 128],
                            wvc[0:65, :],
                        )
                    nc.scalar.copy(
                        v_sb[:, k0:k1, :].rearrange("p a b -> p (a b)"),
                        ps[:, 0 : nch * 66],
                    )
                    yield
                nc.gpsimd.memset(v_sb[:, :, 64:65], 1.0)
                # DVE Schraudolph per-partition bias: c2 = c*LOG2E128 + EXPB
                c2 = rpool.tile([128, NKC], F32, tag="c2", name=f"c2{i}")
                nc.vector.tensor_scalar(
                    c2[:], v_sb[:, :, 65], LOG2E128, EXPB, ALU.mult, ALU.add
                )
                yield
                state[i] = {"xx": xx, "u": ut, "v": v_sb, "c2": c2, "pt": {}}

            def sc_pair(i, jb, kc):
                """One kc-pair of row-tiled score matmuls + their exps."""
                st = state[i]
                xx, ut, v_sb, c2 = st["xx"], st["u"], st["v"], st["c2"]
                pt = st["pt"].setdefault(jb, [None] * NKC)
                tA = sppool.tile([128, QB], F32, tag="sp", name=f"sA{i}_{jb}_{kc}")
                tB = sppool.tile([128, QB], F32, tag="sp", name=f"sB{i}_{jb}_{kc}")
                lA = ut[0:64, kc * 128 : (kc + 1) * 128]
                lB = ut[64:128, (kc + 8) * 128 : (kc + 9) * 128]
                # A/B interleaved: the two row-halves stream concurrently
                for qm in range(QB // 512):
                    q0 = jb * QB + qm * 512
                    sl = slice(qm * 512, (qm + 1) * 512)
                    nc.tensor.matmul(tA[:, sl], lA, xx[0:64, q0 : q0 + 512])
                    nc.tensor.matmul(tB[:, sl], lB, xx[64:128, q0 : q0 + 512])
                pA = ptpool.tile([128, QB], BF, tag="pt", name=f"pA{i}_{jb}_{kc}")
                pB = ptpool.tile([128, QB], BF, tag="pt", name=f"pB{i}_{jb}_{kc}")
                for half, (p, t, c) in enumerate(((pA, tA, kc), (pB, tB, kc + 8))):
                    if kc * 2 + half in DVE_EXP:
                        nc.vector.tensor_scalar(
                            p[:].bitcast(I16),
                            t[:],
                            EXPA,
                            c2[:, c : c + 1],
                            ALU.mult,
                            ALU.add,
                        )
                    else:
                        nc.scalar.activation(
                            p[:], t[:], AF.Exp, bias=v_sb[:, c, 65:66], scale=SCALE
                        )
                pt[kc] = pA
                pt[kc + 8] = pB

            def av_steps(i, jb):
                """Generator: AV matmuls in groups of 4; numerators parked in
                SBUF, denominator rows bounced to DRAM. After the last
                q-chunk of the q-block: one [128,8] reciprocal, bounce back,
                broadcast-read per chunk, GPSIMD multiply, store."""
                v_sb = state[i]["v"]
                pt = state[i]["pt"].pop(jb)
                ots_list = []
                for qm in range(QB // 512):
                    av = avpool.tile([128, 512], F32, tag="av", name=f"av{i}_{jb}_{qm}")
                    for kc in range(NKC):
                        nc.tensor.matmul(
                            av[0:65, :],
                            v_sb[:, kc, 0:65],
                            pt[kc][:, qm * 512 : (qm + 1) * 512],
                            start=(kc == 0),
                            stop=(kc == NKC - 1),
                        )
                        if kc % 4 == 3 and kc < NKC - 1:
                            yield
                    # drain copy emitted immediately after the last AV matmul
                    # so it queues ahead of subsequently-emitted exp work
                    ots = otpool.tile([65, 512], F32, tag="ot", name=f"ot{i}_{jb}_{qm}")
                    if i == HPC - 1:
                        # tail fast path: 1-lane recip straight from the AV
                        # PSUM row so the normalize chain starts immediately
                        r1 = rpool.tile([1, 512], F32, tag="r1", name=f"r1{i}_{jb}_{qm}")
                        nc.vector.reciprocal(r1[:], av[64:65, :])
                        nc.sync.dma_start(
                            rcd[i, jb, qm * 512 : (qm + 1) * 512].rearrange(
                                "(a n) -> a n", a=1
                            ),
                            r1[:],
                        )
                    nc.vector.tensor_copy(ots[:], av[0:65, :])
                    if i < HPC - 1:
                        nc.sync.dma_start(
                            dnd[i, jb, qm * 512 : (qm + 1) * 512].rearrange(
                                "(a n) -> a n", a=1
                            ),
                            ots[64:65, :],
                        )
                    ots_list.append(ots)
                    yield
                import concourse.mybir as mybir

                if i < HPC - 1:
                    den = rpool.tile([128, 8], F32, tag="r", name=f"dn{i}_{jb}")
                    nc.sync.dma_start(
                        den[:], dnd[i, jb].rearrange("(p c) -> p c", c=8)
                    )
                    r8 = rpool.tile([128, 8], F32, tag="r", name=f"rc{i}_{jb}")
                    nc.vector.reciprocal(r8[:], den[:])
                    nc.sync.dma_start(
                        rcd[i, jb].rearrange("(p c) -> p c", c=8), r8[:]
                    )
                    yield
                for qm, ots in enumerate(ots_list):
                    g = jb * (QB // 512) + qm
                    rb = rpool.tile([64, 512], F32, tag="rb", name=f"rb{i}_{jb}_{qm}")
                    nc.sync.dma_start(
                        rb[:],
                        rcd[i, jb, qm * 512 : (qm + 1) * 512]
                        .rearrange("(a n) -> a n", a=1)
                        .to_broadcast((64, 512)),
                    )
                    nc.gpsimd.tensor_tensor(
                        ots[0:64, :], ots[0:64, :], rb[:], mybir.AluOpType.mult
                    )
                    nc.sync.dma_start(
                        outd[i][:, g * 512 : (g + 1) * 512], ots[0:64, :]
                    )
                    yield

            fillers = deque()

            def pump(n):
                while n > 0 and fillers:
                    try:
                        next(fillers[0])
                        n -= 1
                    except StopIteration:
                        fillers.popleft()

            def drain(gen=None):
                while fillers and (gen is None or gen in fillers):
                    pump(1)

            def unit(i, jb):
                # score pairs emitted in bursts of two: fewer PE phase
                # transitions (each score<->AV switch costs a ~300ns bubble)
                for kc in range(0, NKC // 2, 8):
                    for k2 in range(8):
                        sc_pair(i, jb, kc + k2)
                    pump(8 * PUMPS_PER_PAIR)

            # head 0 projections run eagerly; afterwards proj(i+1) + AV trail
            # the score stream as interleaved filler, lagging by one q-block
            g0 = proj_steps(0)
            fillers.append(g0)
            drain(g0)
            unit(0, 0)
            for i in range(HPC):
                if i > 0:
                    fillers.append(av_steps(i - 1, 1))
                    unit(i, 0)
                fillers.append(av_steps(i, 0))
                if i + 1 < HPC:
                    g = proj_steps(i + 1)
                    fillers.append(g)
                    unit(i, 1)
                    drain(g)
                else:
                    unit(i, 1)
            fillers.append(av_steps(HPC - 1, 1))
            drain()

    _split_multi_waits(nc)
    _BUILT = nc
    return nc


def _core_inputs(sequences, wq, bq, wk, bk, wv, bv):
    import ml_dtypes

    bf16 = ml_dtypes.bfloat16
    xh = np.asarray(sequences, dtype=np.float32).reshape(B, S, H, DH)
    wq, bq = np.asarray(wq, np.float32), np.asarray(bq, np.float32)
    wk, bk = np.asarray(wk, np.float32), np.asarray(bk, np.float32)
    wv, bv = np.asarray(wv, np.float32), np.asarray(bv, np.float32)
    in_maps = []
    for c in range(NCORES):
        xt = np.empty((HPC, 65, S), dtype=bf16)
        wu = np.empty((HPC, 65, 64), dtype=bf16)
        wvc = np.zeros((HPC, 65, 66), dtype=bf16)
        for i in range(HPC):
            f = c * HPC + i
            b, h = f // H, f % H
            xt[i, 0:64] = np.ascontiguousarray(xh[b, :, h, :].T).astype(bf16)
            xt[i, 64] = np.float32(1.0)
            # u = Wu^T xt:  Wu = [[ (Wq^T Wk)^T ]; (Wq^T bk)^T ] as [65, 64]
            M = wq[h].T @ wk[h]  # [a, b]
            g = wq[h].T @ bk[h]  # [a]
            wu[i, 0:64] = M.T.astype(bf16)
            wu[i, 64] = g.astype(bf16)
            # V projection rhs: [Wv^T; bv] in cols 0:64, zeros col 64 (ones
            # placeholder), c-weights col 65 (pre-scaled by 1/8)
            wvc[i, 0:64, 0:64] = wv[h].T.astype(bf16)
            wvc[i, 64, 0:64] = bv[h].astype(bf16)
            wc = np.concatenate([wk[h].T @ bq[h], [bq[h] @ bk[h]]]) * SCALE
            wvc[i, :, 65] = wc.astype(bf16)
        in_maps.append({"xt": xt, "wu": wu, "wvc": wvc})
    return in_maps


def _gather(results):
    out = np.empty((B, S, H, DH), np.float32)
    for c in range(NCORES):
        o = np.asarray(results[c]["out"])  # [HPC, 64, S]
        for i in range(HPC):
            f = c * HPC + i
            b, h = f // H, f % H
            out[b, :, h, :] = o[i].T
    return out.reshape(B, S, D)


def kernel(sequences, wq, bq, wk, bk, wv, bv):
    from concourse.bass_utils import run_bass_kernel_spmd

    nc = build()
    in_maps = _core_inputs(sequences, wq, bq, wk, bk, wv, bv)
    res = run_bass_kernel_spmd(nc, in_maps, list(range(NCORES)))
    return _gather(res.results)


# revision 34
# speedup vs baseline: 1.0498x; 1.0035x over previous
"""Multi-head attention (B=4, S=2048, D=768, H=12) on 8 Trainium2 cores.

Sharding: the 48 (batch, head) pairs are data-parallel; each core gets 6.

Per head on one core (matmuls bf16, fp32 PSUM accumulation):
  Host folds the Q/K projections into one: s_qk = x̃_q·u_k + c_k with
    u = Ŵuᵀ x̃  (Ŵu = [[WqᵀWk]; (Wqᵀbk)ᵀ], x̃ = [x; 1], K=65)
    c = ŵcᵀ x̃  (ŵc = [Wkᵀbq; bqᵀbk]/8, folded into the V projection)
  so only u (not q and k) is computed on device, and the per-k score bias
  c rides the exp as a per-partition bias operand.
  uT [128, S]  : u duplicated into both partition halves directly by the
                 projection (two col-tiled matmuls) so score matmuls can
                 row-pair: two K=64 matmuls concurrent in the PE array
                 (lhsT = uT halves, rhs = x duplicated via double-DMA).
  V    [S,64+1]: ones column appended -> AV matmul also produces the
                 softmax denominator. V projection (lhsT = x̃ chunks,
                 rhs = [WvT;bv | 0 | ŵc]) emits V, the ones placeholder,
                 and the c column in one FD=66 matmul per 128 positions.
  scoresT [k,q]: per 128-row k-chunk, [128, 1024] PSUM tiles; score
                 matmuls ordered A,B,A,B so the two row-halves stream
                 concurrently in the PE array.
  P = exp(s/8 + c): split between ACT (exact, activation bias=c) and DVE
                 (Schraudolph bits: int16(s*A + c2) reinterpreted as bf16,
                 ~3% max rel err) so neither engine is the wall.
  out^T [65,512] = sum_k V_aug^T P; row 64 = softmax denominator.
                 Denominator rows bounce through DRAM and return as one
                 [128, 16] tile for a single cheap reciprocal; a 0-stride
                 DMA re-read broadcasts each reciprocal row across
                 partitions; the normalize multiply runs on GPSIMD to keep
                 the DVE free for exp. Output lands in [e, q] layout; the
                 host gather transposes it back.

Scheduling: score-pair emission is interleaved with filler PE work (AV
matmuls of the previous q-block, projections of the next head) via a FIFO
of generators, keeping tensor/ACT/DVE all dense.
"""

import sys
from collections import deque

for _p in ("/opt/trn_rl_repo",):
    if _p not in sys.path:
        sys.path.insert(0, _p)

import numpy as np

B, S, D, H = 4, 2048, 768, 12
DH = 64
NCORES = 8
HPC = (B * H) // NCORES  # 6 heads per core
SCALE = 1.0 / 8.0
NKC = S // 128  # 16 k-chunks
NQB = 2  # q blocks of 1024
QB = S // NQB
PUMPS_PER_PAIR = 5

# Schraudolph exp-as-bf16-bits on the DVE: bits_i16 = round(s*EXPA + c2)
# approximates bf16(exp(s/8 + c)) to ~3% max rel err; offloads the ACT engine.
LOG2E128 = 128.0 / np.log(2.0)
EXPA = LOG2E128 / 8.0  # includes the 1/8 softmax scale
EXPB = (127.0 - 0.0436) * 128.0
# tile indices (kc*2 + half) handled by DVE instead of ACT, per (head, jb)
DVE_EXP = frozenset({1, 3, 5, 7, 9, 11, 13})


def _split_multi_waits(nc):
    """This walrus build rejects >1 sync wait per instruction. Insert
    single-wait NoOps (same engine, so same instruction stream) ahead of
    any instruction carrying several waits."""
    import bass_rust
    import concourse.mybir as mybir

    n_split = 0
    for f in nc.m.functions:
        for bb in f.blocks:
            out = []
            dirty = False
            for inst in bb.instructions:
                si = inst.sync_info
                if si is not None and len(si.on_wait) > 1:
                    waits = list(si.on_wait)
                    for j, w in enumerate(waits[:-1]):
                        nop = mybir.InstNoOp(name=f"{inst.name}-w{j}", ins=[], outs=[])
                        nop.engine = inst.engine
                        nop.sync_info = bass_rust.SyncInfo(on_wait=[w], on_update=[])
                        out.append(nop)
                    si.on_wait = waits[-1:]
                    dirty = True
                    n_split += 1
                out.append(inst)
            if dirty:
                bb.instructions = out
    return n_split


_BUILT = None


def build():
    global _BUILT
    if _BUILT is not None:
        return _BUILT
    import concourse.bass as bass
    import concourse.mybir as mybir
    import concourse.tile as tile

    F32 = mybir.dt.float32
    BF = mybir.dt.bfloat16
    I16 = mybir.dt.int16
    AF = mybir.ActivationFunctionType
    ALU = mybir.AluOpType

    nc = bass.Bass()
    xtd = nc.dram_tensor("xt", [HPC, 65, S], BF, kind="ExternalInput")
    wud = nc.dram_tensor("wu", [HPC, 65, 64], BF, kind="ExternalInput")
    wvcd = nc.dram_tensor("wvc", [HPC, 65, 66], BF, kind="ExternalInput")
    outd = nc.dram_tensor("out", [HPC, 64, S], F32, kind="ExternalOutput")
    dnd = nc.dram_tensor("dnd", [HPC, NQB, QB], F32)  # denominator bounce
    rcd = nc.dram_tensor("rcd", [HPC, NQB, QB], F32)  # reciprocal bounce

    # V-proj psum grouping: 16 chunks in groups of 7/7/2 (66 cols each)
    VGRP = ((0, 7), (7, 14), (14, 16))

    with tile.TileContext(nc) as tc:
        with (
            tc.tile_pool(name="x", bufs=2) as xpool,
            tc.tile_pool(name="w", bufs=2) as wpool,
            tc.tile_pool(name="u", bufs=2) as upool,
            tc.tile_pool(name="v", bufs=2) as vpool,
            tc.tile_pool(name="pt", bufs=2 * NKC * NQB) as ptpool,
            tc.tile_pool(name="ot", bufs=9) as otpool,
            tc.tile_pool(name="r", bufs=3) as rpool,
            tc.tile_pool(name="sp", bufs=3, space="PSUM") as sppool,
            tc.tile_pool(name="avp", bufs=2, space="PSUM") as avpool,
        ):
            state = {}

            def proj_steps(i):
                # u-proj dependencies issue first (and on the sync queue) so
                # head-0 matmuls start as early as possible
                xt = xpool.tile([65, S], BF, tag="xt", name=f"xt{i}")
                nc.sync.dma_start(xt[0:65, :], xtd[i])
                wu = wpool.tile([65, 64], BF, tag="wu", name=f"wu{i}")
                nc.sync.dma_start(wu[0:65, :], wud[i])
                xx = xpool.tile([128, S], BF, tag="xx", name=f"xx{i}")
                nc.gpsimd.dma_start(xx[0:64, :], xtd[i, 0:64])
                nc.gpsimd.dma_start(xx[64:128, :], xtd[i, 0:64])
                wvc = wpool.tile([65, 66], BF, tag="wvc", name=f"wvc{i}")
                nc.gpsimd.dma_start(wvc[0:65, :], wvcd[i])
                yield

                # u projection, duplicated into both partition halves by
                # col-tiled matmul pairs (concurrent in the PE array)
                ut = upool.tile([128, S], BF, tag="u", name=f"u{i}")
                for ch in range(4):
                    ps = sppool.tile([128, 512], F32, tag="sp", name=f"up{i}_{ch}")
                    rhs = xt[0:65, ch * 512 : (ch + 1) * 512]
                    nc.tensor.matmul(ps[0:64, :], wu[0:65, :], rhs)
                    nc.tensor.matmul(ps[64:128, :], wu[0:65, :], rhs)
                    nc.scalar.copy(ut[:, ch * 512 : (ch + 1) * 512], ps[:])
                    yield

                # V projection: per 128-position chunk one FD=66 matmul
                # emitting [V (64) | 0 (ones placeholder) | c], grouped so one
                # PSUM->SBUF copy covers up to 7 chunks.
                v_sb = vpool.tile([128, NKC, 66], BF, tag="v", name=f"v{i}")
                for k0, k1 in VGRP:
                    nch = k1 - k0
                    ps = sppool.tile([128, 462], F32, tag="sp", name=f"vp{i}_{k0}")
                    for j in range(nch):
                        kc = k0 + j
                        nc.tensor.matmul(
                            ps[:, j * 66 : (j + 1) * 66],
                            xt[0:65, kc * 128 : (kc + 1) * 128],
                            wvc[0:65, :],
                        )
                    nc.scalar.copy(
                        v_sb[:, k0:k1, :].rearrange("p a b -> p (a b)"),
                        ps[:, 0 : nch * 66],
                    )
                    yield
                nc.gpsimd.memset(v_sb[:, :, 64:65], 1.0)
                # DVE Schraudolph per-partition bias: c2 = c*LOG2E128 + EXPB
                c2 = rpool.tile([128, NKC], F32, tag="c2", name=f"c2{i}")
                nc.vector.tensor_scalar(
                    c2[:], v_sb[:, :, 65], LOG2E128, EXPB, ALU.mult, ALU.add
                )
                yield
                state[i] = {"xx": xx, "u": ut, "v": v_sb, "c2": c2, "pt": {}}

            def sc_pair(i, jb, kc):
                """One kc-pair of row-tiled score matmuls + their exps."""
                st = state[i]
                xx, ut, v_sb, c2 = st["xx"], st["u"], st["v"], st["c2"]
                pt = st["pt"].setdefault(jb, [None] * NKC)
                tA = sppool.tile([128, QB], F32, tag="sp", name=f"sA{i}_{jb}_{kc}")
                tB = sppool.tile([128, QB], F32, tag="sp", name=f"sB{i}_{jb}_{kc}")
                lA = ut[0:64, kc * 128 : (kc + 1) * 128]
                lB = ut[64:128, (kc + 8) * 128 : (kc + 9) * 128]
                # A/B interleaved: the two row-halves stream concurrently
                for qm in range(QB // 512):
                    q0 = jb * QB + qm * 512
                    sl = slice(qm * 512, (qm + 1) * 512)
                    nc.tensor.matmul(tA[:, sl], lA, xx[0:64, q0 : q0 + 512])
                    nc.tensor.matmul(tB[:, sl], lB, xx[64:128, q0 : q0 + 512])
                pA = ptpool.tile([128, QB], BF, tag="pt", name=f"pA{i}_{jb}_{kc}")
                pB = ptpool.tile([128, QB], BF, tag="pt", name=f"pB{i}_{jb}_{kc}")
                # at the very end of the kernel, keep the DVE queue clear so
                # the tail's drain/reciprocal chain isn't stuck behind exps
                last_unit = i == HPC - 1 and jb == NQB - 1
                for half, (p, t, c) in enumerate(((pA, tA, kc), (pB, tB, kc + 8))):
                    if kc * 2 + half in DVE_EXP and not (last_unit and kc >= 6):
                        nc.vector.tensor_scalar(
                            p[:].bitcast(I16),
                            t[:],
                            EXPA,
                            c2[:, c : c + 1],
                            ALU.mult,
                            ALU.add,
                        )
                    else:
                        nc.scalar.activation(
                            p[:], t[:], AF.Exp, bias=v_sb[:, c, 65:66], scale=SCALE
                        )
                pt[kc] = pA
                pt[kc + 8] = pB

            def av_steps(i, jb):
                """Generator: AV matmuls in groups of 4; numerators parked in
                SBUF, denominator rows bounced to DRAM. After the last
                q-chunk of the q-block: one [128,8] reciprocal, bounce back,
                broadcast-read per chunk, GPSIMD multiply, store."""
                v_sb = state[i]["v"]
                pt = state[i]["pt"].pop(jb)
                ots_list = []
                for qm in range(QB // 512):
                    av = avpool.tile([128, 512], F32, tag="av", name=f"av{i}_{jb}_{qm}")
                    for kc in range(NKC):
                        nc.tensor.matmul(
                            av[0:65, :],
                            v_sb[:, kc, 0:65],
                            pt[kc][:, qm * 512 : (qm + 1) * 512],
                            start=(kc == 0),
                            stop=(kc == NKC - 1),
                        )
                        if kc % 4 == 3 and kc < NKC - 1:
                            yield
                    # drain copy emitted immediately after the last AV matmul
                    # so it queues ahead of subsequently-emitted exp work
                    ots = otpool.tile([65, 512], F32, tag="ot", name=f"ot{i}_{jb}_{qm}")
                    if i == HPC - 1:
                        # tail fast path: 1-lane recip straight from the AV
                        # PSUM row so the normalize chain starts immediately
                        r1 = rpool.tile([1, 512], F32, tag="r1", name=f"r1{i}_{jb}_{qm}")
                        nc.vector.reciprocal(r1[:], av[64:65, :])
                        nc.sync.dma_start(
                            rcd[i, jb, qm * 512 : (qm + 1) * 512].rearrange(
                                "(a n) -> a n", a=1
                            ),
                            r1[:],
                        )
                    nc.vector.tensor_copy(ots[:], av[0:65, :])
                    if i < HPC - 1:
                        nc.sync.dma_start(
                            dnd[i, jb, qm * 512 : (qm + 1) * 512].rearrange(
                                "(a n) -> a n", a=1
                            ),
                            ots[64:65, :],
                        )
                    ots_list.append(ots)
                    yield
                import concourse.mybir as mybir

                if i < HPC - 1:
                    den = rpool.tile([128, 8], F32, tag="r", name=f"dn{i}_{jb}")
                    nc.sync.dma_start(
                        den[:], dnd[i, jb].rearrange("(p c) -> p c", c=8)
                    )
                    r8 = rpool.tile([128, 8], F32, tag="r", name=f"rc{i}_{jb}")
                    nc.vector.reciprocal(r8[:], den[:])
                    nc.sync.dma_start(
                        rcd[i, jb].rearrange("(p c) -> p c", c=8), r8[:]
                    )
                    yield
                for qm, ots in enumerate(ots_list):
                    g = jb * (QB // 512) + qm
                    rb = rpool.tile([64, 512], F32, tag="rb", name=f"rb{i}_{jb}_{qm}")
                    nc.sync.dma_start(
                        rb[:],
                        rcd[i, jb, qm * 512 : (qm + 1) * 512]
                        .rearrange("(a n) -> a n", a=1)
                        .to_broadcast((64, 512)),
                    )
                    nc.gpsimd.tensor_tensor(
                        ots[0:64, :], ots[0:64, :], rb[:], mybir.AluOpType.mult
                    )
                    nc.sync.dma_start(
                        outd[i][:, g * 512 : (g + 1) * 512], ots[0:64, :]
                    )
                    yield

            fillers = deque()

            def pump(n):
                while n > 0 and fillers:
                    try:
                        next(fillers[0])
                        n -= 1
                    except StopIteration:
                        fillers.popleft()

            def drain(gen=None):
                while fillers and (gen is None or gen in fillers):
                    pump(1)

            def unit(i, jb):
                # score pairs emitted in bursts of two: fewer PE phase
                # transitions (each score<->AV switch costs a ~300ns bubble)
                for kc in range(0, NKC // 2, 8):
                    for k2 in range(8):
                        sc_pair(i, jb, kc + k2)
                    pump(8 * PUMPS_PER_PAIR)

            # head 0 projections run eagerly; afterwards proj(i+1) + AV trail
            # the score stream as interleaved filler, lagging by one q-block
            g0 = proj_steps(0)
            fillers.append(g0)
            drain(g0)
            unit(0, 0)
            for i in range(HPC):
                if i > 0:
                    fillers.append(av_steps(i - 1, 1))
                    unit(i, 0)
                fillers.append(av_steps(i, 0))
                if i + 1 < HPC:
                    g = proj_steps(i + 1)
                    fillers.append(g)
                    unit(i, 1)
                    drain(g)
                else:
                    unit(i, 1)
            fillers.append(av_steps(HPC - 1, 1))
            drain()

    _split_multi_waits(nc)
    _BUILT = nc
    return nc


def _core_inputs(sequences, wq, bq, wk, bk, wv, bv):
    import ml_dtypes

    bf16 = ml_dtypes.bfloat16
    xh = np.asarray(sequences, dtype=np.float32).reshape(B, S, H, DH)
    wq, bq = np.asarray(wq, np.float32), np.asarray(bq, np.float32)
    wk, bk = np.asarray(wk, np.float32), np.asarray(bk, np.float32)
    wv, bv = np.asarray(wv, np.float32), np.asarray(bv, np.float32)
    in_maps = []
    for c in range(NCORES):
        xt = np.empty((HPC, 65, S), dtype=bf16)
        wu = np.empty((HPC, 65, 64), dtype=bf16)
        wvc = np.zeros((HPC, 65, 66), dtype=bf16)
        for i in range(HPC):
            f = c * HPC + i
            b, h = f // H, f % H
            xt[i, 0:64] = np.ascontiguousarray(xh[b, :, h, :].T).astype(bf16)
            xt[i, 64] = np.float32(1.0)
            # u = Wu^T xt:  Wu = [[ (Wq^T Wk)^T ]; (Wq^T bk)^T ] as [65, 64]
            M = wq[h].T @ wk[h]  # [a, b]
            g = wq[h].T @ bk[h]  # [a]
            wu[i, 0:64] = M.T.astype(bf16)
            wu[i, 64] = g.astype(bf16)
            # V projection rhs: [Wv^T; bv] in cols 0:64, zeros col 64 (ones
            # placeholder), c-weights col 65 (pre-scaled by 1/8)
            wvc[i, 0:64, 0:64] = wv[h].T.astype(bf16)
            wvc[i, 64, 0:64] = bv[h].astype(bf16)
            wc = np.concatenate([wk[h].T @ bq[h], [bq[h] @ bk[h]]]) * SCALE
            wvc[i, :, 65] = wc.astype(bf16)
        in_maps.append({"xt": xt, "wu": wu, "wvc": wvc})
    return in_maps


def _gather(results):
    out = np.empty((B, S, H, DH), np.float32)
    for c in range(NCORES):
        o = np.asarray(results[c]["out"])  # [HPC, 64, S]
        for i in range(HPC):
            f = c * HPC + i
            b, h = f // H, f % H
            out[b, :, h, :] = o[i].T
    return out.reshape(B, S, D)


def kernel(sequences, wq, bq, wk, bk, wv, bv):
    from concourse.bass_utils import run_bass_kernel_spmd

    nc = build()
    in_maps = _core_inputs(sequences, wq, bq, wk, bk, wv, bv)
    res = run_bass_kernel_spmd(nc, in_maps, list(range(NCORES)))
    return _gather(res.results)
